# revision 15
# baseline (speedup 1.0000x reference)
"""Trainium2 Bass kernel for nn_L2MLoRAqkv (MoE-routed LoRA QKV projection).

Math (per batch b, expert i = idx[b,0]):
    qkv = x @ W.T + bias
    qkv[:, :D]  += (x @ A_q[i]) @ B_q[i] * SCALE
    qkv[:, -D:] += (x @ A_v[i]) @ B_v[i] * SCALE

Strategy: data-parallel over the batch dim (1 batch per NeuronCore, 8 cores).
On the host we gather each batch's expert and fold the rank-8 LoRA update
into the (transposed) projection weight in float64:
    W_eff[b] = W.T; W_eff[:, :D] += A_q[i] @ B_q[i]; W_eff[:, -D:] += A_v[i] @ B_v[i]
so the device kernel is a single dense GEMM per core:
    Y[4096, 3072] = X[4096, 1024] @ W_eff[1024, 3072] + bias
X is pre-transposed on the host ([D, T], K-major) so both matmul operands
load with K on SBUF partitions via fast contiguous DMAs.

Matmul operands are bf16: same 1 cycle/row PE rate as f32r, but the
LDWEIGHTS runs via fast-weight-load and hides fully under the moving
stream (f32r exposes ~15ns/MM).  The accumulation stays fp32 in PSUM;
quantization error ~0.2% against a 2e-2 gate.  Output is stored bf16
(halves store traffic + drain tail) and upcast on the host.
"""

import os
import sys

import numpy as np

for _p in ("/opt/trn_rl_repo",):
    if _p not in sys.path and os.path.isdir(_p):
        sys.path.insert(0, _p)

B = 8          # batches == cores
T = 4096       # tokens per batch
D = 1024       # model dim (contraction K)
N3 = 3072      # qkv output dim
P = 128        # SBUF partitions
NT = 512       # n-tile (one fp32 PSUM bank)
CHUNK = 512    # token chunk streamed per DMA group
KT = D // P        # 8 k-tiles
NN = N3 // NT      # 6 n-tiles
TT = CHUNK // P    # 4 token sub-tiles per chunk
SCALE = 8.0 / 8.0

MM_DTYPE = "bfloat16"
OUT_DTYPE = "bfloat16"

_NC_CACHE = {}


def _build(mm_dtype_name=MM_DTYPE, tokens=T):
    import concourse.tile as tile
    from concourse import bacc, mybir

    nchunk = tokens // CHUNK
    mmdt = getattr(mybir.dt, mm_dtype_name)
    outdt = getattr(mybir.dt, OUT_DTYPE)
    f32 = mybir.dt.float32

    nc = bacc.Bacc(
        "TRN2",
        target_bir_lowering=False,
        debug=False,
        enable_asserts=False,
        num_devices=B,
    )
    xt = nc.dram_tensor("xt", [D, tokens], mmdt, kind="ExternalInput").ap()
    weff = nc.dram_tensor("weff", [D, N3], mmdt, kind="ExternalInput").ap()
    biasr = nc.dram_tensor("biasr", [P, N3], mmdt, kind="ExternalInput").ap()
    y = nc.dram_tensor("y", [tokens, N3], outdt, kind="ExternalOutput").ap()

    # 3D views with the k-tile dim explicit, for single-descriptor DMAs.
    xt3 = xt.rearrange("(k p) t -> p k t", p=P)        # [P, KT, tokens]
    weff3 = weff.rearrange("(k p) n -> p k n", p=P)    # [P, KT, N3]

    HEAD = 2  # chunks in the n-outer head (halves the early weff demand rate)

    with tile.TileContext(nc) as tc:
        with tc.tile_pool(name="const", bufs=1) as const_pool, \
             tc.tile_pool(name="xin", bufs=4) as xin_pool, \
             tc.tile_pool(name="outp", bufs=8) as out_pool, \
             tc.tile_pool(name="ps", bufs=8, space="PSUM") as psum_pool:

            # W_eff resident in SBUF as 8 k-slices side by side: [128, 8*3072].
            w_sb = const_pool.tile([P, KT * N3], mmdt)
            w_sb3 = w_sb.rearrange("p (k n) -> p k n", n=N3)  # [P, KT, N3]

            def alloc_chunk():
                xc = xin_pool.tile([P, KT * CHUNK], mmdt, tag="xc", name="xc")
                return xc, xc.rearrange("p (k c) -> p k c", c=CHUNK)

            # --- startup-critical loads, k-granular and ring-interleaved ---
            # The first matmul group consumes xc0[k] + weff[k, n=0] pairs in k
            # order; alternate which ring carries x vs w so the pairs land in
            # consumption order while the two rings share the ~358GB/s HBM
            # port.  (DIRECT2D issue is ~0.6us/descriptor, so everything
            # non-critical below is batched into single 3D descriptors.)
            xc0, xc0_3 = alloc_chunk()
            for k in range(KT):
                eng_x, eng_w = (nc.sync, nc.scalar) if k % 2 == 0 else (nc.scalar, nc.sync)
                eng_x.dma_start(xc0_3[:, k, :], xt3[:, k, 0:CHUNK])
                eng_w.dma_start(w_sb3[:, k, 0:NT], weff3[:, k, 0:NT])

            # Chunk 1 next (the head interleaves chunks 0+1), one descriptor.
            xc1, xc1_3 = alloc_chunk()
            nc.sync.dma_start(xc1_3[:, :, :], xt3[:, :, CHUNK : 2 * CHUNK])

            # Remaining W n-slices: one 3D descriptor per n, alternating rings,
            # in n order so the head's n-outer matmul groups unblock in arrival
            # order.  bias (bf16, SWDGE ring) is only needed by the first drain.
            bias_sb = const_pool.tile([P, N3], mmdt)
            for n in range(1, NN):
                eng = nc.scalar if n % 2 else nc.sync
                eng.dma_start(
                    w_sb3[:, :, n * NT : (n + 1) * NT],
                    weff3[:, :, n * NT : (n + 1) * NT],
                )
                if n == 1:
                    nc.gpsimd.dma_start(bias_sb[:], biasr[:])

            store_ctr = [0]
            store_rings = [None, None, None]  # filled below

            def drain(ps, c, t, n, last=False):
                ob = out_pool.tile([P, NT], outdt, tag="ob", name="ob")
                ydst = y[c * CHUNK + t * P : c * CHUNK + (t + 1) * P,
                         n * NT : (n + 1) * NT]
                bslice = bias_sb[:, n * NT : (n + 1) * NT]
                if last:
                    # Final group: split the add+store in half so the store of
                    # the first half overlaps the add of the second, shortening
                    # the post-matmul critical chain.
                    h = NT // 2
                    for i in range(2):
                        nc.vector.tensor_add(
                            ob[:, i * h : (i + 1) * h],
                            ps[:, i * h : (i + 1) * h],
                            bslice[:, i * h : (i + 1) * h],
                        )
                        store_rings[i].dma_start(
                            ydst[:, i * h : (i + 1) * h], ob[:, i * h : (i + 1) * h]
                        )
                    return
                nc.vector.tensor_add(ob[:], ps[:], bslice)
                # Stores round-robin over both HWDGE rings + the SWDGE ring --
                # but the SWDGE ring's multi-us completion drain would sit on
                # the critical path at the end, so the last chunk only uses
                # the HWDGE rings.
                if c == nchunk - 1:
                    eng = store_rings[store_ctr[0] % 2]
                else:
                    eng = store_rings[store_ctr[0] % 3]
                store_ctr[0] += 1
                eng.dma_start(ydst, ob[:])

            store_rings[0] = nc.sync
            store_rings[1] = nc.scalar
            store_rings[2] = nc.gpsimd

            def do_group(xc, c, t, n, last=False):
                ps = psum_pool.tile([P, NT], f32, tag="ps", name="ps")
                for k in range(KT):
                    nc.tensor.matmul(
                        ps[:],
                        lhsT=xc[:, k * CHUNK + t * P : k * CHUNK + (t + 1) * P],
                        rhs=w_sb[:, k * N3 + n * NT : k * N3 + (n + 1) * NT],
                        start=(k == 0),
                        stop=(k == KT - 1),
                    )
                drain(ps, c, t, n, last=last)

            # Head: chunks 0+1 n-outer, so matmul groups unblock in weff
            # DMA-arrival order and never outrun the ~358GB/s HBM port.
            head_xcs = [xc0, xc1]
            for n in range(NN):
                for c in range(HEAD):
                    for t in range(TT):
                        do_group(head_xcs[c], c, t, n)

            # Remaining chunks: weff fully resident; single-descriptor loads,
            # alternating rings per chunk.
            for c in range(HEAD, nchunk):
                xc, xc3 = alloc_chunk()
                eng = nc.scalar if c % 2 else nc.sync
                eng.dma_start(xc3[:, :, :], xt3[:, :, c * CHUNK : (c + 1) * CHUNK])
                for t in range(TT):
                    for n in range(NN):
                        last = c == nchunk - 1 and t == TT - 1 and n == NN - 1
                        do_group(xc, c, t, n, last=last)
    nc.compile()
    return nc


def _get_nc(mm_dtype_name=MM_DTYPE, tokens=T):
    key = (mm_dtype_name, tokens)
    if key not in _NC_CACHE:
        _NC_CACHE[key] = _build(mm_dtype_name, tokens)
    return _NC_CACHE[key]


def _prep_in_maps(inputs):
    import ml_dtypes

    mm_np = ml_dtypes.bfloat16 if MM_DTYPE == "bfloat16" else np.float32

    x = np.asarray(inputs["x"], dtype=np.float32)
    weight = np.asarray(inputs["weight"], dtype=np.float32)
    bias = np.asarray(inputs["bias"], dtype=np.float32)
    aq = np.asarray(inputs["A_q_pool"], dtype=np.float32)
    bq = np.asarray(inputs["B_q_pool"], dtype=np.float32)
    av = np.asarray(inputs["A_v_pool"], dtype=np.float32)
    bv = np.asarray(inputs["B_v_pool"], dtype=np.float32)
    idx = np.asarray(inputs["idx"]).reshape(B, -1)[:, 0].astype(np.int64)

    wt64 = weight.T.astype(np.float64)  # [D, N3]
    biasr = np.ascontiguousarray(np.broadcast_to(bias, (P, N3))).astype(mm_np)
    xts = x.transpose(0, 2, 1)  # [B, D, T] strided view

    in_maps = []
    for b in range(B):
        i = int(idx[b])
        weff = wt64.copy()
        weff[:, :D] += SCALE * (aq[i].astype(np.float64) @ bq[i].astype(np.float64))
        weff[:, N3 - D:] += SCALE * (av[i].astype(np.float64) @ bv[i].astype(np.float64))
        in_maps.append({
            "xt": np.ascontiguousarray(xts[b]).astype(mm_np),
            "weff": weff.astype(mm_np),
            "biasr": biasr,
        })
    return in_maps


def _run(in_maps, trace=False, **kwargs):
    from concourse.bass_utils import run_bass_kernel_spmd

    nc = _get_nc()
    return run_bass_kernel_spmd(
        nc, in_maps, core_ids=list(range(B)), trace=trace, **kwargs
    )


def kernel(**inputs):
    res = _run(_prep_in_maps(inputs), trace=False)
    return np.stack([r["y"] for r in res.results], axis=0).astype(np.float32)
